# revision 17
# baseline (speedup 1.0000x reference)
"""Multi-head attention TRN2 kernel: 8-core head-sharded tensor parallelism.

Full inputs in, full output out. Each core computes 2 of the 16 heads:
QKV projection (its column slice), flash-style attention, and a partial
out-projection against its row slice of Wo. Host sums the 8 partials + bo.

Per-core device program (identical SPMD; per-core weight slices via in_maps):
  phase A: QKV proj  Q^T/K^T/V^T [128, 4096] = W_slice^T-stationary @ x^T
  phase B: PE-transpose V^T -> Vaug [k, d] tiles with a ones column
           (ones column makes the ctx matmul also emit softmax row-sums)
  phase C: per (batch, q-block, head): scores^T = K^T-tiles @ Q^T (PSUM),
           exp on 1024-wide tiles (ACT), ctx^T accumulation (PE),
           1/rowsum via DVE + PE-broadcast, normalize into ctx2T
  phase D: out-proj y[t, fo] = ctx2T-token-tile-stationary @ Wo_slice,
           DMA straight from PSUM to DRAM partial output

All matmul operands are float32r (TF32-like, ~1.5e-4 rel err, 1 cyc/row).
"""
import sys

sys.path.insert(0, "/opt/trn_rl_repo")

from contextlib import ExitStack

import numpy as np

import concourse.bass as bass
import concourse.tile as tile
from concourse import bacc, mybir
from concourse.bass_utils import run_bass_kernel_spmd
from concourse.masks import make_identity

f32 = mybir.dt.float32
f32r = mybir.dt.float32r
EXP = mybir.ActivationFunctionType.Exp

N_CORES = 8
B, S, F = 2, 2048, 1024
H = 16                 # heads total
DK = F // H            # 64
HPC = H // N_CORES     # 2 heads per core
CF = HPC * DK          # 128 = per-core slice of features
T = B * S              # 4096 tokens
NKT = S // 128         # 16 key tiles per sequence
NQB = S // 512         # 4 q-blocks per sequence
NTT = 512 // 128       # 4 token-tiles per q-block


def build_program():
    nc = bacc.Bacc("TRN2", target_bir_lowering=False, debug=False,
                   num_devices=N_CORES)

    xt_d = nc.dram_tensor("xT", [F, T], f32r, kind="ExternalInput").ap()
    wqkv_d = nc.dram_tensor("Wqkv", [F, 3 * CF], f32r, kind="ExternalInput").ap()
    bqkv_d = nc.dram_tensor("bqkv", [3 * CF, 1], f32, kind="ExternalInput").ap()
    wo_d = nc.dram_tensor("Wo", [CF, F], f32r, kind="ExternalInput").ap()
    yp_d = nc.dram_tensor("yp", [T, F], f32, kind="ExternalOutput").ap()

    with tile.TileContext(nc) as tc, ExitStack() as ctx:
        const = ctx.enter_context(tc.tile_pool(name="const", bufs=1))
        big = ctx.enter_context(tc.tile_pool(name="big", bufs=1))
        xpool = ctx.enter_context(tc.tile_pool(name="xpool", bufs=4))
        etp = ctx.enter_context(tc.tile_pool(name="etp", bufs=4))
        small = ctx.enter_context(tc.tile_pool(name="small", bufs=3))
        ypool = ctx.enter_context(tc.tile_pool(name="ypool", bufs=8))

        # ---- constants ----
        wqkv_sb = const.tile([128, 8, 3 * CF], f32r)  # [f-part, f-tile, out-col]
        nc.sync.dma_start(wqkv_sb, wqkv_d.rearrange("(a p) n -> p a n", p=128))
        wo_sb = const.tile([128, F], f32r)
        nc.sync.dma_start(wo_sb, wo_d)
        btiles = []
        for p3 in range(3):
            bt = const.tile([128, 1], f32, name=f"bias{p3}", tag=f"bias{p3}")
            nc.sync.dma_start(bt, bqkv_d[p3 * CF:(p3 + 1) * CF, :])
            btiles.append(bt)
        ident_f = const.tile([128, 128], f32)
        make_identity(nc, ident_f)
        ident_r = const.tile([128, 128], f32r)
        nc.vector.tensor_copy(ident_r, ident_f)
        ones_f = const.tile([128, 64], f32)
        nc.vector.memset(ones_f, 1.0)
        ones_r = const.tile([1, 64], f32r)
        nc.vector.tensor_copy(ones_r, ones_f[0:1, :])

        # ---- persistent activations ----
        qt_sb = big.tile([128, T], f32r)       # [2 heads x 64 d, tokens]
        kt_sb = big.tile([128, T], f32r)
        vt_sb = big.tile([128, T], f32r)
        vaug_sb = big.tile([128, B, NKT, HPC, 65], f32r)  # [k-part, b, kt, h, d+1]
        ctx2t_sb = big.tile([128, B, S], f32r)  # [2 heads x 64 d, b, tokens]

        qkvt = [qt_sb, kt_sb, vt_sb]

        # ======== phase A: QKV projections ========
        with tc.tile_pool(name="qkv_ps", bufs=6, space="PSUM") as qkv_ps:
            TBLK = 1024
            warm_ps = qkv_ps.tile([128, 512], f32, tag="warm", name="warm_ps", bufs=1)
            for tb in range(T // TBLK):
                pqs = [qkv_ps.tile([128, 512], f32, tag="pq",
                                   name=f"pq{tb}_{i}") for i in range(6)]
                for a in range(8):
                    xt_t = xpool.tile([128, TBLK], f32r, tag="xt",
                                      name=f"xt{tb}_{a}")
                    nc.sync.dma_start(
                        xt_t,
                        xt_d[a * 128:(a + 1) * 128, tb * TBLK:(tb + 1) * TBLK])
                    for p3 in range(3):
                        for half in range(TBLK // 512):
                            nc.tensor.matmul(
                                pqs[p3 * 2 + half],
                                wqkv_sb[:, a, p3 * CF:(p3 + 1) * CF],
                                xt_t[:, half * 512:(half + 1) * 512],
                                start=(a == 0), stop=(a == 7))
                    for _ in range(2):
                        nc.tensor.matmul(
                            warm_ps, wqkv_sb[:, a, 0:128],
                            xt_t[:, 0:512], start=True, stop=True)
                for p3 in range(3):
                    for half in range(TBLK // 512):
                        dst = qkvt[p3][:, tb * TBLK + half * 512:
                                       tb * TBLK + (half + 1) * 512]
                        nc.vector.tensor_scalar_add(dst, pqs[p3 * 2 + half],
                                                    btiles[p3])

        # ======== phase B: V transpose into Vaug ========
        # ones column for all (b, kt, h) in one strided copy
        nc.vector.tensor_copy(
            vaug_sb[:, :, :, :, 64:65],
            ones_f[:, 0:B * NKT * HPC].rearrange(
                "p (b k h o) -> p b k h o", b=B, k=NKT, h=HPC))
        with tc.tile_pool(name="vt_ps", bufs=2, space="PSUM") as vt_ps:
            for b in range(B):
                for kt_i in range(NKT):
                    pv = vt_ps.tile([128, 128], f32r, tag="pv")
                    nc.tensor.transpose(
                        pv, vt_sb[:, b * S + kt_i * 128:b * S + (kt_i + 1) * 128],
                        ident_r)
                    nc.vector.tensor_copy(
                        vaug_sb[:, b, kt_i, :, 0:64],
                        pv.rearrange("p (h d) -> p h d", h=HPC))

        # ======== phases C+D: flash attention + out-projection ========
        fs_ps = ctx.enter_context(
            tc.tile_pool(name="fs_ps", bufs=3, space="PSUM"))
        pc_ps = ctx.enter_context(
            tc.tile_pool(name="pc_ps", bufs=1, space="PSUM"))
        op_ps = ctx.enter_context(
            tc.tile_pool(name="op_ps", bufs=1, space="PSUM"))

        # steps: one per (b, qb, h, pair j of 2 ktiles)
        steps = []
        for b in range(B):
            for qb in range(NQB):
                for h in range(HPC):
                    for j in range(NKT // 2):
                        steps.append((b, qb, h, j))

        score_ps = {}
        exp_sb = {}
        ctx_ps = {}

        def emit_scores(i):
            b, qb, h, j = steps[i]
            pss = fs_ps.tile([128, 1024], f32, tag="fs", name=f"fs{i}")
            for u in range(2):
                kt_i = 2 * j + u
                nc.tensor.matmul(
                    pss[:, u * 512:(u + 1) * 512],
                    kt_sb[h * 64:(h + 1) * 64,
                          b * S + kt_i * 128:b * S + (kt_i + 1) * 128],
                    qt_sb[h * 64:(h + 1) * 64,
                          b * S + qb * 512:b * S + (qb + 1) * 512],
                    start=True, stop=True)
            score_ps[i] = pss

        def emit_exp(i):
            et = etp.tile([128, 1024], f32r, tag="et", name=f"et{i}")
            nc.scalar.activation(et, score_ps[i], EXP)
            exp_sb[i] = et

        def emit_ctx(i):
            b, qb, h, j = steps[i]
            if j == 0:
                ctx_ps[(b, qb, h)] = pc_ps.tile(
                    [65, 512], f32, tag="pc", name=f"pc{i}")
            pctx = ctx_ps[(b, qb, h)]
            for u in range(2):
                kt_i = 2 * j + u
                nc.tensor.matmul(
                    pctx, vaug_sb[:, b, kt_i, h, :],
                    exp_sb[i][:, u * 512:(u + 1) * 512],
                    start=(kt_i == 0), stop=(kt_i == NKT - 1))
            if j == NKT // 2 - 1:
                emit_norm(b, qb, h)
                if h == HPC - 1:
                    for tt in range(NTT):
                        op_queue.append((b, qb, tt))

        def emit_norm(b, qb, h):
            pctx = ctx_ps.pop((b, qb, h))
            rs = small.tile([1, 512], f32, tag="rs", name=f"rs{b}{qb}{h}")
            nc.vector.tensor_copy(rs, pctx[64:65, :])
            rcp = small.tile([1, 512], f32, tag="rcp", name=f"rcp{b}{qb}{h}")
            nc.vector.reciprocal_approx_fast(rcp, rs)
            rcp_r = small.tile([1, 512], f32r, tag="rcpr", name=f"rcpr{b}{qb}{h}")
            nc.vector.tensor_copy(rcp_r, rcp)
            pb = op_ps.tile([64, 512], f32, tag="op", name=f"pb{b}{qb}{h}")
            nc.tensor.matmul(pb, ones_r, rcp_r, start=True, stop=True)
            cu = small.tile([64, 512], f32, tag="cu", name=f"cu{b}{qb}{h}")
            nc.vector.tensor_copy(cu, pctx[0:64, :])
            nc.vector.tensor_mul(
                ctx2t_sb[h * 64:(h + 1) * 64, b,
                         qb * 512:(qb + 1) * 512],
                cu, pb)

        op_queue = []

        def drain_outproj(n=1):
            for _ in range(n):
                if not op_queue:
                    return
                b, qb, tt = op_queue.pop(0)
                tok0 = qb * 512 + tt * 128
                ysb = ypool.tile([128, 1024], f32, tag="ysb",
                                 name=f"ysb{b}{qb}{tt}")
                for wh in range(2):
                    py = op_ps.tile([128, 512], f32, tag="op",
                                    name=f"py{b}{qb}{tt}{wh}")
                    nc.tensor.matmul(
                        py, ctx2t_sb[:, b, tok0:tok0 + 128],
                        wo_sb[:, wh * 512:(wh + 1) * 512],
                        start=True, stop=True)
                    nc.vector.tensor_copy(ysb[:, wh * 512:(wh + 1) * 512], py)
                nc.sync.dma_start(
                    yp_d[b * S + tok0:b * S + tok0 + 128, :], ysb)

        emit_scores(0)
        emit_scores(1)
        emit_exp(0)
        for i in range(2, len(steps)):
            emit_scores(i)
            emit_exp(i - 1)
            emit_ctx(i - 2)
            if i % 2 == 0:
                drain_outproj(1)
            nc.tensor.matmul(
                score_ps[i - 2][:, 0:512],
                kt_sb[:, 0:128], qt_sb[:, 0:512],
                start=True, stop=True)
        emit_exp(len(steps) - 1)
        emit_ctx(len(steps) - 2)
        emit_ctx(len(steps) - 1)
        drain_outproj(len(op_queue))

    nc.compile()
    return nc


_NC = None


def kernel(x, Wq, bq, Wk, bk, Wv, bv, Wo, bo):
    global _NC
    x = np.asarray(x, dtype=np.float32)
    Wq = np.asarray(Wq, dtype=np.float32)
    Wk = np.asarray(Wk, dtype=np.float32)
    Wv = np.asarray(Wv, dtype=np.float32)
    Wo = np.asarray(Wo, dtype=np.float32)
    bq = np.asarray(bq, dtype=np.float32)
    bk = np.asarray(bk, dtype=np.float32)
    bv = np.asarray(bv, dtype=np.float32)
    bo = np.asarray(bo, dtype=np.float32)

    if _NC is None:
        _NC = build_program()
    nc = _NC

    sc = 1.0 / np.sqrt(np.float32(DK))
    xT = np.ascontiguousarray(x.reshape(T, F).T)  # [F, T]

    in_maps = []
    for c in range(N_CORES):
        sl = slice(c * CF, (c + 1) * CF)
        wqkv = np.concatenate([Wq[:, sl] * sc, Wk[:, sl], Wv[:, sl]], axis=1)
        bqkv = np.concatenate([bq[sl] * sc, bk[sl], bv[sl]])
        in_maps.append({
            "xT": xT,
            "Wqkv": np.ascontiguousarray(wqkv),
            "bqkv": np.ascontiguousarray(bqkv),
            "Wo": np.ascontiguousarray(Wo[sl, :]),
        })

    res = run_bass_kernel_spmd(nc, in_maps, list(range(N_CORES)))
    y = res.results[0]["yp"].astype(np.float64)
    for c in range(1, N_CORES):
        y += res.results[c]["yp"]
    y = (y + bo).astype(np.float32)
    return y.reshape(B, S, F)
